# revision 18
# baseline (speedup 1.0000x reference)
"""Trainium2 Bass kernel for nn_CumulativeFlattenedLinear (segment_reduce).

Per window of S=64 timesteps: per-timestep C->O projection (weights zero for
the first n_discard steps) + causal cumsum within the window, plus bias.

Strategy (data-parallel over batch, 1 batch element per core), fp16 I/O:
  - x and y cross HBM as fp16 (host converts) with 1KB contiguous runs:
    partition = 512-element time chunk, 2 supertiles. Loads/stores are split
    across the two HWDGE queues (SP + ACT) to overlap both directions.
  - Per window: 6 fp16 PE transposes build x^T (k=(c,v')) in PSUM, one ACT
    copy moves it to SBUF; 6 triangular "intra" matmuls write the window's
    (s,o)-major PSUM region; block totals accumulate into a single shared
    128-col PSUM "pre" region seeded with bias by a K=1 matmul, so prefix
    sums come out of PSUM accumulation for free.
  - One DVE broadcast-add per window evicts intra+prefix to the fp16 output
    tile; GPSIMD fills the leading n_discard positions with bias.
"""
import numpy as np

import concourse.bass as bass
import concourse.tile as tile
from concourse import bacc, mybir
from concourse.bass_utils import run_bass_kernel_spmd

F16 = mybir.dt.float16
F32 = mybir.dt.float32

B, C, T, O = 8, 16, 131072, 16
P = 128
CH = 512                  # time elems per partition per supertile (1KB fp16)
NST = T // (P * CH)       # 2 supertiles
V = 8                     # sub-block length
NU = 8                    # sub-blocks per window
S = NU * V                # 64
NW = CH // S              # windows per partition row = 8

_cache = {}

# eviction reads prefix totals straight from PSUM (2 PSUM operands on DVE);
# flip to True if hardware rejects that and route through an SBUF copy
PRE_VIA_SBUF = True


def _build_nc(first_u):
    DU = NU - first_u          # active sub-blocks (trailing)
    NPRE = DU - 1              # blocks contributing prefix totals
    fill_s = first_u * V       # s < fill_s -> output = bias

    nc = bacc.Bacc("TRN2", target_bir_lowering=False, debug=False)
    x_d = nc.dram_tensor("x", (C, T), F16, kind="ExternalInput")
    wi_d = nc.dram_tensor("w_intra", (P, DU * 128), F16, kind="ExternalInput")
    wp_d = nc.dram_tensor("w_pre", (P, max(NPRE, 1) * 128), F16,
                          kind="ExternalInput")
    ident_d = nc.dram_tensor("ident", (P, P), F16, kind="ExternalInput")
    ones_d = nc.dram_tensor("ones_k1", (1, P), F16, kind="ExternalInput")
    brow_d = nc.dram_tensor("biasrow", (1, P), F16, kind="ExternalInput")
    bfill_d = nc.dram_tensor("bias_fill", (P, max(O * NW * fill_s, 1)), F16,
                             kind="ExternalInput")
    y_d = nc.dram_tensor("y", (O, T), F16, kind="ExternalOutput")

    xv = x_d.ap().rearrange("c (st p hs) -> st p c hs", st=NST, p=P, hs=CH)
    yv = y_d.ap().rearrange("o (st p hs) -> st p o hs", st=NST, p=P, hs=CH)

    PREBASE = DU * 128         # psum tile: [intra (s>=fill_s) | pre region]

    with tile.TileContext(nc) as tc:
        with (
            tc.tile_pool(name="const", bufs=1) as cp,
            tc.tile_pool(name="io", bufs=2) as io,
            tc.tile_pool(name="shf", bufs=3) as shf,
            tc.tile_pool(name="tsb", bufs=3) as tsb,
            tc.tile_pool(name="mid", bufs=3) as mid,
            tc.tile_pool(name="psT", bufs=2, space="PSUM") as psT,
            tc.tile_pool(name="psW", bufs=2, space="PSUM") as psW,
        ):
            w_intra = cp.tile([P, DU * 128], F16, name="w_intra")
            nc.sync.dma_start(w_intra[:], wi_d.ap())
            w_pre = cp.tile([P, max(NPRE, 1) * 128], F16, name="w_pre")
            nc.sync.dma_start(w_pre[:], wp_d.ap())
            ident = cp.tile([P, P], F16, name="ident")
            nc.sync.dma_start(ident[:], ident_d.ap())
            ones = cp.tile([1, P], F16, name="ones_k1")
            nc.sync.dma_start(ones[:], ones_d.ap())
            brow = cp.tile([1, P], F16, name="biasrow")
            nc.sync.dma_start(brow[:], brow_d.ap())
            bfill = cp.tile([P, max(O * NW * fill_s, 1)], F16, name="bfill")
            nc.sync.dma_start(bfill[:], bfill_d.ap())

            # all input loads issued up-front, split across both HWDGE queues
            xins = []
            for st in range(NST):
                xin = io.tile([P, C * CH], F16, name=f"xin{st}", tag="xin")
                xr = xin[:].rearrange("p (c hs) -> p c hs", c=C)
                nc.sync.dma_start(xr[0:64], xv[st, 0:64])
                nc.scalar.dma_start(xr[64:128], xv[st, 64:128])
                xins.append(xin)
            outs = [io.tile([P, O * CH], F16, name=f"out{st}", tag="out")
                    for st in range(NST)]

            # bias fill for s < fill_s: one batched GPSIMD op per supertile,
            # issued early so it's off the per-window critical path
            if fill_s:
                for st in range(NST):
                    outf = outs[st][:].rearrange(
                        "p (o w s) -> p o w s", o=O, w=NW
                    )[:, :, :, 0:fill_s]
                    nc.gpsimd.tensor_copy(
                        outf.bitcast(F32),
                        bfill[:].rearrange(
                            "p (o w s) -> p o w s", o=O, w=NW
                        ).bitcast(F32),
                    )

            state = {}

            def front(st, w, widx):
                xin = xins[st]
                # shuffle window columns to (u, c, v) blocks; alternate the
                # engine so neither DVE nor GPSIMD becomes the bottleneck
                sh = shf.tile([P, DU * 128], F16, name="shuf", tag="shuf")
                src = xin[:].rearrange(
                    "p (c w u v) -> w p u c v", c=C, w=NW, u=NU, v=V
                )[w][:, first_u:NU]
                eng = nc.vector if widx % 2 == 0 else nc.gpsimd
                eng.tensor_copy(
                    sh[:].rearrange(
                        "p (u c v) -> p u c v", u=DU, c=C, v=V
                    ).bitcast(F32),
                    src.bitcast(F32),
                )
                pt = psT.tile([P, DU * 128], F16, name="pt", tag="pt")
                for du in range(DU):
                    nc.tensor.transpose(
                        pt[:, du * 128:(du + 1) * 128],
                        sh[:, du * 128:(du + 1) * 128],
                        ident[:],
                    )
                ts = tsb.tile([P, DU * 128], F16, name="ts", tag="ts")
                nc.scalar.copy(ts[:].bitcast(F32), pt[:].bitcast(F32))
                state[(st, w)] = ts

            def back(st, w):
                ts = state.pop((st, w))
                out_sb = outs[st]
                pw = psW.tile([P, PREBASE + P], F32, name="pw", tag="pw")
                for du in range(DU):
                    lo = du * 128
                    nc.tensor.matmul(
                        pw[:, lo:lo + 128],
                        ts[:, du * 128:(du + 1) * 128],
                        w_intra[:, du * 128:(du + 1) * 128],
                        start=True, stop=True, skip_group_check=True,
                    )
                # seed pre region with bias AFTER the intra matmuls: start=True
                # clears has_written bank-wide, and the pre region shares a
                # PSUM bank with the last intra blocks
                nc.tensor.matmul(
                    pw[:, PREBASE:PREBASE + P], ones[:], brow[:],
                    start=True, stop=(NPRE == 0), skip_group_check=True,
                )
                for pu in range(NPRE):
                    nc.tensor.matmul(
                        pw[:, PREBASE:PREBASE + P],
                        ts[:, pu * 128:(pu + 1) * 128],
                        w_pre[:, pu * 128:(pu + 1) * 128],
                        start=False, stop=(pu == NPRE - 1),
                        skip_group_check=True,
                    )
                if PRE_VIA_SBUF:
                    pre_sb = mid.tile([P, P], F32, name="pre_sb", tag="pre_sb")
                    nc.scalar.copy(pre_sb[:], pw[:, PREBASE:PREBASE + P])
                    pre_ap = pre_sb[:, first_u * O:]
                else:
                    pre_ap = pw[:, PREBASE + first_u * O:PREBASE + P]
                # eviction: out[(o, s)] = intra + prefix, fp16
                out4 = out_sb[:].rearrange(
                    "p (o w u v) -> w p o u v", o=O, w=NW, u=NU, v=V
                )[w][:, :, first_u:NU]
                in1 = pw[:, 0:DU * 128].rearrange(
                    "p (u v o) -> p o u v", u=DU, v=V, o=O
                )
                in2 = pre_ap.rearrange(
                    "p (u o) -> p o u", u=DU, o=O
                ).unsqueeze(3).broadcast_to([P, O, DU, V])
                nc.vector.tensor_add(out4, in1, in2)

            def store(st):
                orr = outs[st][:].rearrange("p (o hs) -> p o hs", o=O)
                nc.scalar.dma_start(yv[st, 0:64], orr[0:64])
                nc.sync.dma_start(yv[st, 64:128], orr[64:128])

            wins = [(st, w) for st in range(NST) for w in range(NW)]
            pending = None
            for widx, stw in enumerate(wins):
                front(*stw, widx)
                if pending is not None:
                    back(*pending)
                    if pending[1] == NW - 1:
                        store(pending[0])
                pending = stw
            back(*pending)
            store(pending[0])
    nc.compile()
    return nc


def _host_constants(weight, bias, n_discard, n_keep):
    Swin = n_discard + n_keep
    assert Swin == S and n_discard % V == 0
    first_u = n_discard // V
    DU = NU - first_u
    NPRE = DU - 1
    fill_s = first_u * V

    w = weight.reshape(O, C, n_keep).transpose(2, 1, 0).astype(np.float32)
    w_full = np.concatenate(
        [np.zeros((n_discard, C, O), np.float32), w], axis=0
    )  # (S, C, O)

    # w_intra[k=(c,vp), du*128 + v*16 + o] = w_full[u*8+vp, c, o] if vp<=v
    blk = np.stack([w_full[(first_u + du) * V:(first_u + du + 1) * V]
                    for du in range(DU)])          # (DU, V, C, O)
    tri = np.zeros((DU, C, V, V, O), np.float32)   # (du, c, vp, v, o)
    vp = np.arange(V)
    for v in range(V):
        tri[:, :, vp <= v, v, :] = blk.transpose(0, 2, 1, 3)[:, :, vp <= v]
    w_intra = tri.reshape(DU, C * V, V * O).transpose(1, 0, 2).reshape(
        P, DU * 128)

    # w_pre[k=(c,vp), pu*128 + ut*16 + o] = w_full[u*8+vp, c, o] if ut>u
    pre = np.zeros((max(NPRE, 1), C, V, NU, O), np.float32)
    for pu in range(NPRE):
        u = first_u + pu
        pre[pu, :, :, u + 1:, :] = blk[pu].transpose(1, 0, 2)[:, :, None, :]
    w_pre = pre.reshape(max(NPRE, 1), C * V, NU * O).transpose(1, 0, 2).reshape(
        P, max(NPRE, 1) * 128)

    bias32 = bias.astype(np.float32)
    consts = {
        "w_intra": np.ascontiguousarray(w_intra).astype(np.float16),
        "w_pre": np.ascontiguousarray(w_pre).astype(np.float16),
        "ident": np.eye(P, dtype=np.float16),
        "ones_k1": np.ones((1, P), np.float16),
        "biasrow": np.tile(bias32, NU)[None, :].astype(np.float16),
        "bias_fill": np.ascontiguousarray(
            np.tile(bias32[:, None], (1, NW * max(fill_s, 1))).reshape(1, -1)
            * np.ones((P, 1), np.float32)
        ).astype(np.float16),
    }
    return consts, first_u


def _run(inputs, trace=False):
    x = np.asarray(inputs["x"], dtype=np.float32)
    weight = np.asarray(inputs["weight"], dtype=np.float32)
    bias = np.asarray(inputs["bias"], dtype=np.float32)
    n_discard = int(inputs["n_discard"])
    n_keep = int(inputs["n_keep"])
    assert x.shape == (B, C, T) and weight.shape == (O, C * n_keep)

    consts, first_u = _host_constants(weight, bias, n_discard, n_keep)
    key = ("nc", first_u)
    if key not in _cache:
        _cache[key] = _build_nc(first_u)
    nc = _cache[key]

    x16 = x.astype(np.float16)
    in_maps = []
    for b in range(B):
        m = dict(consts)
        m["x"] = np.ascontiguousarray(x16[b])
        in_maps.append(m)
    res = run_bass_kernel_spmd(nc, in_maps, list(range(B)), trace=trace)
    y = np.stack([res.results[b]["y"] for b in range(B)], axis=0)
    return y.astype(np.float32), res


def kernel(**inputs):
    y, _ = _run(inputs, trace=False)
    return y
